# revision 2
# baseline (speedup 1.0000x reference)
"""Self-contained 8-NeuronCore Trainium2 kernel for a 16-head MHA layer.

Problem (hardcoded): x [2, 2048, 1024] f32, torch-style Linear weights
Wq/Wk/Wv/Wo [1024, 1024] + biases. y = MHA(x) with 16 heads of dim 64.

Sharding: tensor-parallel over heads. Core c owns heads {2c, 2c+1}, i.e.
feature slice F = [128c, 128c+128). Each core:
  phase 1  qT/kT/vT = W[F] @ x.T + b          (fp32r matmuls, [128, 4096])
  phase 1b v_pv     = per-token-chunk transpose of vT with a ones column
                      appended per head (bf16) -> PV lhsT [128, 65]
  phase 2  per (batch, head): S.T = k.T-chunks x q (fp32r), exp on ACT with
           fused 1/8 scale into bf16 tiles; PV matmul [v|1].T @ expS.T gives
           numerator rows 0..63 and the softmax denominator in row 64;
           normalize via DVE reciprocal + K=1 matmul partition-broadcast;
           the partial output projection is interleaved per 512-token chunk.
  phase 3  zT_partial = Wo[:, F].T.T @ O.T    (fp32r)   [1024, 4096]
Host sums the 8 partial zT outputs, adds bo, transposes back.
"""

import numpy as np

import concourse.bass as bass
import concourse.tile as tile
from concourse import mybir
from concourse.bass_utils import run_bass_kernel_spmd
from concourse.masks import make_identity

# ---------------------------------------------------------------- constants
B = 2
NSEQ = 2048
NIN = 1024
H = 16
DH = 64
P = 128
NTOK = B * NSEQ            # 4096
KO = NIN // P              # 8 contraction chunks for the projections
NCORES = 8
HPC = H // NCORES          # 2 heads per core
TCH = 512                  # projection token chunk (psum bank)
QH = 1024                  # q columns per S.T psum tile / exp instruction
KT = NSEQ // P             # 16 key-token tiles per (batch, head)
EXP_BUFS = 28              # live expS tiles (16 held by PV + fill-ahead)

F32 = mybir.dt.float32
F32R = mybir.dt.float32r
BF16 = mybir.dt.bfloat16
AF = mybir.ActivationFunctionType

_SCALE = 0.125             # 1/sqrt(DH)


# ------------------------------------------------- walrus workaround (env)
# The walrus build in this environment rejects Drain instructions carrying
# more than one semaphore wait ("Too many sync wait commands").  Split the
# final Tile drain into one single-wait drain per semaphore.
def _patched_drain_and_barrier(self, tick_clock, wait_clock):
    nc = self.nc
    drain_inst = nc.sync.drain()
    wait_clock.add_sem_waits(
        drain_inst.ins, tile.ScopedClock({None: tick_clock.global_clock})
    )
    si = drain_inst.ins.sync_info
    waits = list(si.on_wait) if si is not None else []
    if len(waits) > 1:
        drain_inst.ins.sync_info = mybir.SyncInfo(
            on_wait=[waits[0]], on_update=list(si.on_update)
        )
        for w in waits[1:]:
            extra = nc.sync.drain()
            extra.ins.sync_info = mybir.SyncInfo(on_wait=[w], on_update=[])
    nc.all_engine_barrier()
    popped = nc._tile_sem_poison_stack.pop()
    assert popped is self._sem_poison
    nc.clear_and_free_semaphores(list(self.sems.allocated().values()))
    nc.all_engine_barrier()


def _install_drain_patch():
    if getattr(tile.TileContext, "_drain_patch_installed", False):
        return
    tile.TileContext._drain_and_barrier = _patched_drain_and_barrier
    tile.TileContext._drain_patch_installed = True


def _split_multi_waits(nc):
    """Same walrus limitation, general form: every instruction may carry at
    most one semaphore wait.  Move extra waits onto same-engine NoOps placed
    immediately before the instruction (engines execute their stream in
    order, so semantics are preserved)."""
    k = 0
    for f in nc.m.functions:
        for blk in f.blocks:
            lst = blk.instructions
            i = 0
            while i < len(lst):
                inst = lst[i]
                si = inst.sync_info
                waits = list(si.on_wait) if si is not None else []
                if len(waits) > 1:
                    for w in waits[:-1]:
                        nop = mybir.InstNoOp(
                            name=f"waitsplit-{k}", engine=inst.engine
                        )
                        k += 1
                        nop.sync_info = mybir.SyncInfo(on_wait=[w], on_update=[])
                        nc.register_instruction(nop)
                        lst.insert(i, nop)
                        i += 1
                    inst.sync_info = mybir.SyncInfo(
                        on_wait=[waits[-1]], on_update=list(si.on_update)
                    )
                i += 1
    return k


# ------------------------------------------------------------ device kernel
def _emit(tc, xT, wq, wk, wv, bqkv, wo, ones64, zT):
    nc = tc.nc
    
    xT_r = xT.rearrange("(ko ki) t -> ki ko t", ki=P)

    with (
        tc.tile_pool(name="const", bufs=1) as const,
        tc.tile_pool(name="persist", bufs=1) as persist,
        tc.tile_pool(name="xin", bufs=2) as xin,
        tc.tile_pool(name="vtmp", bufs=3) as vtmp,
        tc.tile_pool(name="exps", bufs=EXP_BUFS) as exps,
        tc.tile_pool(name="work", bufs=3) as work,
        tc.tile_pool(name="zout", bufs=4) as zout,
        tc.tile_pool(name="ps_big", bufs=2, space="PSUM") as ps_big,
        tc.tile_pool(name="ps_pv", bufs=2, space="PSUM") as ps_pv,
        tc.tile_pool(name="ps_mm", bufs=2, space="PSUM") as ps_mm,
    ):
        # ---- constants
        wq_sb = const.tile([P, KO, P], F32R)
        wk_sb = const.tile([P, KO, P], F32R)
        wv_sb = const.tile([P, KO, P], F32R)
        for w_sb, w in ((wq_sb, wq), (wk_sb, wk), (wv_sb, wv)):
            nc.sync.dma_start(out=w_sb, in_=w.rearrange("(ko ki) m -> ki ko m", ki=P))
        wo_sb = const.tile([P, NIN], F32R)
        nc.sync.dma_start(out=wo_sb, in_=wo)
        bias_sb = const.tile([P, 3], F32)
        nc.sync.dma_start(out=bias_sb, in_=bqkv)
        ones_sb = const.tile([1, DH], F32R)
        nc.sync.dma_start(out=ones_sb, in_=ones64)
        ident = const.tile([P, P], F32)
        make_identity(nc, ident)

        qT = persist.tile([P, NTOK], BF16)
        kT = persist.tile([P, NTOK], BF16)
        # PV lhsT: per token chunk, per head: 64 v-dims + a ones column.
        v_pv = persist.tile([P, NTOK // P, 2 * (DH + 1)], BF16)
        nc.vector.memset(v_pv, 1.0)
        OT = persist.tile([P, NTOK], F32R)

        # ---- phase 1: QKV projections (feature-major layouts)
        for n in range(NTOK // TCH):
            tsl = slice(n * TCH, (n + 1) * TCH)
            xt = xin.tile([P, KO, TCH], F32R, tag="xt")
            nc.sync.dma_start(out=xt, in_=xT_r[:, :, tsl])
            for pi, w_sb in enumerate((wq_sb, wk_sb, wv_sb)):
                ps = ps_mm.tile([P, TCH], F32, tag="mm")
                for ko in range(KO):
                    nc.tensor.matmul(
                        ps,
                        w_sb[:, ko],
                        xt[:, ko],
                        start=(ko == 0),
                        stop=(ko == KO - 1),
                    )
                # evacuate on ACT only while it is still idle (first batch's
                # chunks, before attention exp work begins); DVE afterwards
                on_act = n < (NTOK // TCH) // 2
                if pi < 2:
                    dst = qT if pi == 0 else kT
                    if on_act:
                        nc.scalar.activation(
                            dst[:, tsl], ps, AF.Identity, bias=bias_sb[:, pi : pi + 1]
                        )
                    else:
                        nc.vector.tensor_add(
                            dst[:, tsl], ps,
                            bias_sb[:, pi : pi + 1].to_broadcast((P, TCH)),
                        )
                else:
                    # v: bias-add into a transient tile, then transpose into v_pv
                    vt = vtmp.tile([P, TCH], F32, tag="vt")
                    if on_act:
                        nc.scalar.activation(
                            vt, ps, AF.Identity, bias=bias_sb[:, 2:3]
                        )
                    else:
                        nc.vector.tensor_add(
                            vt, ps, bias_sb[:, 2:3].to_broadcast((P, TCH))
                        )
                    pst = ps_mm.tile([P, TCH], F32, tag="mm")
                    for t2 in range(TCH // P):
                        nc.tensor.matmul(
                            pst[:, t2 * P : (t2 + 1) * P],
                            vt[:, t2 * P : (t2 + 1) * P],
                            ident,
                            is_transpose=True,
                            start=True,
                            stop=True,
                        )
                    for t2 in range(TCH // P):
                        t = n * (TCH // P) + t2
                        if on_act:
                            nc.scalar.activation(
                                v_pv[:, t, 0:DH],
                                pst[:, t2 * P : t2 * P + DH],
                                AF.Identity,
                            )
                            nc.scalar.activation(
                                v_pv[:, t, DH + 1 : 2 * DH + 1],
                                pst[:, t2 * P + DH : t2 * P + 2 * DH],
                                AF.Identity,
                            )
                        else:
                            nc.vector.tensor_copy(
                                out=v_pv[:, t, 0:DH],
                                in_=pst[:, t2 * P : t2 * P + DH],
                            )
                            nc.vector.tensor_copy(
                                out=v_pv[:, t, DH + 1 : 2 * DH + 1],
                                in_=pst[:, t2 * P + DH : t2 * P + 2 * DH],
                            )

        # ---- phase 2: attention, both heads packed per (batch, q-chunk).
        # The two heads' S.T matmuls write one psum tile as [h0-512 | h1-512]
        # columns; being adjacent instructions on disjoint PE row groups
        # (rows 0-63 vs 64-127, from the operands' base partitions) they run
        # concurrently in the array, recovering the K=64 half-array waste.
        # One exp then covers both heads.  PV + normalize run per head, and
        # the partial projection follows per finished 512-token chunk.
        for b in range(B):
            for j in range(NSEQ // TCH):
                qoff = b * NSEQ + j * TCH
                etiles = []
                for kt in range(KT):
                    ps = ps_big.tile([P, QH], F32, tag="st")
                    ksl = slice(b * NSEQ + kt * P, b * NSEQ + (kt + 1) * P)
                    for hl in range(HPC):
                        hsl = slice(DH * hl, DH * hl + DH)
                        nc.tensor.matmul(
                            ps[:, hl * TCH : (hl + 1) * TCH],
                            kT[hsl, ksl],
                            qT[hsl, qoff : qoff + TCH],
                            start=True,
                            stop=True,
                        )
                    e = exps.tile([P, QH], BF16, tag="e")
                    nc.scalar.activation(e, ps, AF.Exp, scale=_SCALE)
                    etiles.append(e)

                dst = slice(qoff, qoff + TCH)
                # both heads' PV accumulators run interleaved so they finish
                # together and the two normalize chains overlap
                pvps = {}
                for hl in (1, 0):
                    pvps[hl] = ps_pv.tile(
                        [P, TCH], F32, tag="pv", name=f"pv_{b}_{j}_{hl}"
                    )
                for kt in range(KT):
                    for hl in (1, 0):
                        nc.tensor.matmul(
                            pvps[hl][0 : DH + 1, :],
                            v_pv[:, b * KT + kt, hl * (DH + 1) : (hl + 1) * (DH + 1)],
                            etiles[kt][:, hl * TCH : (hl + 1) * TCH],
                            start=(kt == 0),
                            stop=(kt == KT - 1),
                        )
                for hl in (1, 0):      # h1 first: its OT shift-DMA overlaps h0
                    ps = pvps[hl]
                    rec = work.tile([1, TCH], F32R, tag="rec")
                    with nc.allow_low_precision(
                        reason="f32r is bit-identical to f32; PE rounds on read"
                    ):
                        nc.vector.reciprocal(rec, ps[DH : DH + 1, :])
                    # partition-broadcast: recB[d, q] = ones[d] * rec[q] via a
                    # K=1 matmul (PE), then evacuate to SBUF for the multiply.
                    psb = ps_mm.tile([P, TCH], F32, tag="mm")
                    nc.tensor.matmul(
                        psb[0:DH, :], ones_sb, rec, start=True, stop=True
                    )
                    recB = work.tile([DH, TCH], F32, tag="recB")
                    nc.vector.tensor_copy(out=recB, in_=psb[0:DH, :])
                    if hl == 0:
                        nc.vector.tensor_mul(OT[0:DH, dst], ps[0:DH, :], recB)
                    else:
                        tmpO = work.tile([DH, TCH], F32R, tag="tmpO")
                        nc.vector.tensor_mul(tmpO, ps[0:DH, :], recB)
                        nc.sync.dma_start(out=OT[DH:P, dst], in_=tmpO)
                # ---- partial output projection for this 512-token chunk;
                # spreads proj compute and the zT output DMA across the whole
                # attention phase (small final tail).
                for co in range(NIN // P):
                    pz = ps_mm.tile([P, TCH], F32, tag="mm")
                    nc.tensor.matmul(
                        pz,
                        wo_sb[:, co * P : (co + 1) * P],
                        OT[:, qoff : qoff + TCH],
                        start=True,
                        stop=True,
                    )
                    zsb = zout.tile([P, TCH], F32, tag="z")
                    if b == B - 1 and j == NSEQ // TCH - 1 and co % 2:
                        nc.scalar.activation(zsb, pz, AF.Identity)
                    else:
                        nc.vector.tensor_copy(out=zsb, in_=pz)
                    nc.sync.dma_start(
                        out=zT[co * P : (co + 1) * P, qoff : qoff + TCH],
                        in_=zsb,
                    )


def _build_nc(repeat=1):
    _install_drain_patch()
    nc = bass.Bass("TRN2", target_bir_lowering=False, debug=False, num_devices=NCORES)
    xT = nc.dram_tensor("xT", [NIN, NTOK], F32R, kind="ExternalInput").ap()
    wq = nc.dram_tensor("wq", [NIN, P], F32R, kind="ExternalInput").ap()
    wk = nc.dram_tensor("wk", [NIN, P], F32R, kind="ExternalInput").ap()
    wv = nc.dram_tensor("wv", [NIN, P], F32R, kind="ExternalInput").ap()
    bqkv = nc.dram_tensor("bqkv", [P, 3], F32, kind="ExternalInput").ap()
    wo = nc.dram_tensor("wo", [P, NIN], F32R, kind="ExternalInput").ap()
    ones64 = nc.dram_tensor("ones64", [1, DH], F32R, kind="ExternalInput").ap()
    zT = nc.dram_tensor("zT", [NIN, NTOK], F32, kind="ExternalOutput").ap()
    with tile.TileContext(nc, num_cores=NCORES) as tc:
        for _ in range(repeat):
            _emit(tc, xT, wq, wk, wv, bqkv, wo, ones64, zT)
    _split_multi_waits(nc)
    return nc


_NC_CACHE = None


def _get_nc():
    global _NC_CACHE
    if _NC_CACHE is None:
        _NC_CACHE = _build_nc()
    return _NC_CACHE


# -------------------------------------------------------------- host wrapper
def _in_maps(x, Wq, bq, Wk, bk, Wv, bv, Wo):
    xTh = np.ascontiguousarray(x.reshape(NTOK, NIN).T.astype(np.float32))
    maps = []
    for c in range(NCORES):
        F = slice(P * c, P * (c + 1))
        maps.append(
            {
                "xT": xTh,
                "wq": np.ascontiguousarray(Wq[F].T),
                "wk": np.ascontiguousarray(Wk[F].T),
                "wv": np.ascontiguousarray(Wv[F].T),
                "bqkv": np.ascontiguousarray(
                    np.stack([bq[F], bk[F], bv[F]], axis=1)
                ),
                "wo": np.ascontiguousarray(Wo[:, F].T),
                "ones64": np.ones((1, DH), np.float32),
            }
        )
    return maps


def kernel(x, Wq, bq, Wk, bk, Wv, bv, Wo, bo, **run_kwargs):
    x = np.asarray(x, np.float32)
    maps = _in_maps(
        x,
        np.asarray(Wq, np.float32),
        np.asarray(bq, np.float32),
        np.asarray(Wk, np.float32),
        np.asarray(bk, np.float32),
        np.asarray(Wv, np.float32),
        np.asarray(bv, np.float32),
        np.asarray(Wo, np.float32),
    )
    nc = _get_nc()
    res = run_bass_kernel_spmd(nc, maps, list(range(NCORES)), **run_kwargs)
    acc = res.results[0]["zT"].astype(np.float32)
    for c in range(1, NCORES):
        acc = acc + res.results[c]["zT"]
    z = acc.T + np.asarray(bo, np.float32)[None, :]
    out = np.ascontiguousarray(z.reshape(B, NSEQ, NIN), dtype=np.float32)
    if run_kwargs:
        return out, res
    return out

